# revision 3
# baseline (speedup 1.0000x reference)
"""Distributed Trainium2 (8 NeuronCores) kernel for a GCN layer:

    xw  = x @ W
    msg = edge_attr[:,None] * xw[src]
    agg = segment_sum(msg, dst, N) + b
    out = relu(gamma * (agg-mean)/sqrt(var+eps) + beta)   (BatchNorm, training stats)

Strategy (nodes sharded by dst across 8 cores, x replicated):
  - The W transform commutes with the weighted scatter-add, so each core
    aggregates raw x features per dst node and applies W once per 128-node
    window:  aggx[n,:] = sum_e attr_e * x[src_e,:]  ->  agg = aggx @ W.
  - Edges are grouped host-side by (core, 128-node dst window), padded to
    128-edge chunks.  Per chunk the scatter-add is a matmul with a weighted
    one-hot matrix A[e, n] = attr_e * (dstl_e == n), built in one dual-op
    vector instruction (is_equal then mult against an iota row).
  - The x[src] gather uses the POOL dma_gather engine.  Its cost is per-row,
    so rows are node PAIRS (256 feats, bf16): the gather index is src//2 and
    the even/odd selection is folded into the one-hot by offsetting the
    dst-local index by 128*(src&1) in a 256-wide window; the two pair halves
    feed two accumulating matmuls.  Pair indices (< 32000) also fit the
    int16 index format of dma_gather.
  - The post-aggregation bias b cancels inside BatchNorm (mean shifts by b)
    and is dropped.  BN statistics are per-feature sums/sumsq with features
    on partitions, reduced over nodes on the free dim (scalar-engine
    accum_out), all-reduced across cores (1KB collective), and applied fused
    with ReLU in one scalar-engine activation per slab.  The output stays
    transposed [128 feat, 8000 nodes] on device; the host transposes.
"""

import sys

for _p in ("/opt/trn_rl_repo",):
    if _p not in sys.path:
        sys.path.insert(0, _p)

import numpy as np
import ml_dtypes

N = 64000
E = 640000
D = 128
NCORES = 8
NPC = N // NCORES          # 8000 nodes per core
WIN = 128                  # dst-window width (PSUM partitions)
NW = (NPC + WIN - 1) // WIN  # 63 windows per core (last one 64 nodes)
GW = 4                     # windows per dma_gather group
BN_EPS = 1e-5

_BF16 = ml_dtypes.bfloat16
_build_cache = {}


def _window_groups():
    groups = []
    w0 = 0
    while w0 < NW:
        groups.append((w0, min(w0 + GW, NW)))
        w0 += GW
    return groups


def _pack_idx_block(idx_i16):
    """Wrap a gather's int16 index list: idx i -> partition i%16, col i//16,
    replicated across the 8 blocks of 16 partitions (Q7 core replicas)."""
    n = idx_i16.shape[0]
    block = idx_i16.reshape(n // 16, 16).T
    return np.tile(block, (8, 1))


def _build(nchw):
    import concourse.bacc as bacc
    import concourse.tile as tile
    from concourse import mybir

    nchunk = NW * nchw                 # chunks per core
    slots = nchunk * 128               # padded edge slots per core
    idxcols = slots // 16

    nc = bacc.Bacc("TRN2", target_bir_lowering=False, debug=False,
                   enable_asserts=False, num_devices=NCORES,
                   num_swdge_queues=4)

    t_table = nc.dram_tensor("table", [N // 2, 2 * D], mybir.dt.bfloat16,
                             kind="ExternalInput")
    t_idxs = nc.dram_tensor("idxs", [128, idxcols], mybir.dt.int16,
                            kind="ExternalInput")
    t_dstlp = nc.dram_tensor("dstlp", [128, nchunk], mybir.dt.float32,
                             kind="ExternalInput")
    t_attr = nc.dram_tensor("attr", [128, nchunk], mybir.dt.float32,
                            kind="ExternalInput")
    t_iota = nc.dram_tensor("iota", [128, 2 * D], mybir.dt.float32,
                            kind="ExternalInput")
    t_w = nc.dram_tensor("wt", [D, D], mybir.dt.bfloat16, kind="ExternalInput")
    t_gamma = nc.dram_tensor("gamma", [128, 1], mybir.dt.float32,
                             kind="ExternalInput")
    t_beta = nc.dram_tensor("beta", [128, 1], mybir.dt.float32,
                            kind="ExternalInput")
    t_out = nc.dram_tensor("outT", [128, NPC], mybir.dt.float32,
                           kind="ExternalOutput")
    t_ccin = nc.dram_tensor("cc_in", [128, 2], mybir.dt.float32)
    t_ccout = nc.dram_tensor("cc_out", [128, 2], mybir.dt.float32,
                             addr_space="Shared")

    groups = _window_groups()

    with tile.TileContext(nc) as tc:
        with (
            tc.tile_pool(name="const", bufs=1) as constp,
            tc.tile_pool(name="agg", bufs=1) as aggp,
            tc.tile_pool(name="xg", bufs=2) as xgp,
            tc.tile_pool(name="amat", bufs=4) as ap_,
            tc.tile_pool(name="axt", bufs=2) as axtp,
            tc.tile_pool(name="stat", bufs=1) as statp,
            tc.tile_pool(name="psA", bufs=2, space="PSUM") as psA,
            tc.tile_pool(name="psO", bufs=2, space="PSUM") as psO,
        ):
            iota_t = constp.tile([128, 2 * D], mybir.dt.float32)
            nc.sync.dma_start(out=iota_t[:], in_=t_iota.ap())
            w_t = constp.tile([D, D], mybir.dt.bfloat16)
            nc.sync.dma_start(out=w_t[:], in_=t_w.ap())
            gamma_t = constp.tile([128, 1], mybir.dt.float32)
            nc.sync.dma_start(out=gamma_t[:], in_=t_gamma.ap())
            beta_t = constp.tile([128, 1], mybir.dt.float32)
            nc.sync.dma_start(out=beta_t[:], in_=t_beta.ap())
            dstlp_t = constp.tile([128, nchunk], mybir.dt.float32)
            nc.sync.dma_start(out=dstlp_t[:], in_=t_dstlp.ap())
            attr_t = constp.tile([128, nchunk], mybir.dt.float32)
            nc.sync.dma_start(out=attr_t[:], in_=t_attr.ap())
            idx_t = constp.tile([128, idxcols], mybir.dt.int16)
            nc.sync.dma_start(out=idx_t[:], in_=t_idxs.ap())

            agg_sb = aggp.tile([128, NW * WIN], mybir.dt.float32)
            out_sb = aggp.tile([128, NW * WIN], mybir.dt.float32)
            sums_t = statp.tile([128, NW], mybir.dt.float32)
            sumsq_t = statp.tile([128, NW], mybir.dt.float32)
            sq_scr = statp.tile([128, WIN], mybir.dt.float32)
            zero1 = statp.tile([128, 1], mybir.dt.float32)
            nc.vector.memset(zero1[:], 0.0)

            for gi, (w0, w1) in enumerate(groups):
                gchunks = (w1 - w0) * nchw
                nidx = gchunks * 128
                xg = xgp.tile([128, GW * nchw, 2 * D], mybir.dt.bfloat16,
                              tag="xg")
                nc.gpsimd.dma_gather(
                    out_ap=xg[:, :gchunks, :],
                    in_ap=t_table.ap(),
                    idxs_ap=idx_t[:, w0 * nchw * 8:w1 * nchw * 8],
                    num_idxs=nidx,
                    num_idxs_reg=nidx,
                    elem_size=2 * D,
                    queue_num=gi % 4,
                    single_packet=False,
                )
                for w in range(w0, w1):
                    ps_aggx = psA.tile([128, WIN], mybir.dt.float32)
                    for c in range(nchw):
                        cc = w * nchw + c
                        s = (w - w0) * nchw + c
                        a_t = ap_.tile([128, 2 * D], mybir.dt.bfloat16,
                                       tag="amat")
                        nc.vector.tensor_scalar(
                            out=a_t[:],
                            in0=iota_t[:],
                            scalar1=dstlp_t[:, cc:cc + 1],
                            scalar2=attr_t[:, cc:cc + 1],
                            op0=mybir.AluOpType.is_equal,
                            op1=mybir.AluOpType.mult,
                        )
                        nc.tensor.matmul(
                            out=ps_aggx[:],
                            lhsT=xg[:, s, 0:D],
                            rhs=a_t[:, 0:WIN],
                            start=(c == 0), stop=False,
                        )
                        nc.tensor.matmul(
                            out=ps_aggx[:],
                            lhsT=xg[:, s, D:2 * D],
                            rhs=a_t[:, D:D + WIN],
                            start=False, stop=(c == nchw - 1),
                        )
                    aggx_bf = axtp.tile([128, WIN], mybir.dt.bfloat16,
                                        tag="axt")
                    nc.vector.tensor_copy(out=aggx_bf[:], in_=ps_aggx[:])
                    ps_out = psO.tile([128, WIN], mybir.dt.float32)
                    nc.tensor.matmul(out=ps_out[:], lhsT=w_t[:],
                                     rhs=aggx_bf[:], start=True, stop=True)
                    v = min(WIN, NPC - w * WIN)
                    nc.scalar.activation(
                        out=agg_sb[:, w * WIN:w * WIN + v],
                        in_=ps_out[:, :v],
                        func=mybir.ActivationFunctionType.Copy,
                        accum_out=sums_t[:, w:w + 1],
                    )
                    nc.scalar.activation(
                        out=sq_scr[:, :v],
                        in_=ps_out[:, :v],
                        func=mybir.ActivationFunctionType.Square,
                        bias=zero1[:, 0:1],
                        accum_out=sumsq_t[:, w:w + 1],
                    )

            # ---- BN statistics: local reduce, cross-core all-reduce ----
            stat2 = statp.tile([128, 2], mybir.dt.float32)
            nc.vector.reduce_sum(stat2[:, 0:1], sums_t[:],
                                 axis=mybir.AxisListType.X)
            nc.vector.reduce_sum(stat2[:, 1:2], sumsq_t[:],
                                 axis=mybir.AxisListType.X)
            nc.sync.dma_start(out=t_ccin.ap(), in_=stat2[:])
            nc.gpsimd.collective_compute(
                "AllReduce", mybir.AluOpType.add,
                replica_groups=[list(range(NCORES))],
                ins=[t_ccin.ap().opt()],
                outs=[t_ccout.ap().opt()],
            )
            tot = statp.tile([128, 2], mybir.dt.float32)
            nc.sync.dma_start(out=tot[:], in_=t_ccout.ap())

            mean = statp.tile([128, 1], mybir.dt.float32)
            nc.vector.tensor_scalar_mul(mean[:], tot[:, 0:1], 1.0 / N)
            ex2 = statp.tile([128, 1], mybir.dt.float32)
            nc.vector.tensor_scalar_mul(ex2[:], tot[:, 1:2], 1.0 / N)
            msq = statp.tile([128, 1], mybir.dt.float32)
            nc.vector.tensor_tensor(out=msq[:], in0=mean[:], in1=mean[:],
                                    op=mybir.AluOpType.mult)
            var = statp.tile([128, 1], mybir.dt.float32)
            nc.vector.tensor_tensor(out=var[:], in0=ex2[:], in1=msq[:],
                                    op=mybir.AluOpType.subtract)
            var_eps = statp.tile([128, 1], mybir.dt.float32)
            nc.vector.tensor_scalar_add(var_eps[:], var[:], BN_EPS)
            sd = statp.tile([128, 1], mybir.dt.float32)
            nc.scalar.activation(out=sd[:], in_=var_eps[:],
                                 func=mybir.ActivationFunctionType.Sqrt,
                                 bias=zero1[:, 0:1])
            rinv = statp.tile([128, 1], mybir.dt.float32)
            nc.vector.reciprocal(rinv[:], sd[:])
            scale = statp.tile([128, 1], mybir.dt.float32)
            nc.vector.tensor_tensor(out=scale[:], in0=gamma_t[:], in1=rinv[:],
                                    op=mybir.AluOpType.mult)
            mscale = statp.tile([128, 1], mybir.dt.float32)
            nc.vector.tensor_tensor(out=mscale[:], in0=mean[:], in1=scale[:],
                                    op=mybir.AluOpType.mult)
            shift = statp.tile([128, 1], mybir.dt.float32)
            nc.vector.tensor_tensor(out=shift[:], in0=beta_t[:], in1=mscale[:],
                                    op=mybir.AluOpType.subtract)

            # ---- normalize + affine + relu, in 8 slabs ----
            slab = 1000
            for s0 in range(0, NPC, slab):
                s1 = min(s0 + slab, NPC)
                nc.scalar.activation(
                    out=out_sb[:, s0:s1],
                    in_=agg_sb[:, s0:s1],
                    func=mybir.ActivationFunctionType.Relu,
                    scale=scale[:, 0:1],
                    bias=shift[:, 0:1],
                )
            nc.sync.dma_start(out=t_out.ap(), in_=out_sb[:, :NPC])

    nc.compile()
    return nc


def _prep(x, edge_index, edge_attr):
    """Host-side shard/pad/pack. Returns (nchw, per-core in_maps extras)."""
    src = edge_index[0].astype(np.int64)
    dst = edge_index[1].astype(np.int64)
    attr = edge_attr.astype(np.float32)

    core = dst // NPC
    wloc = (dst % NPC) // WIN
    gwin = core * NW + wloc                      # global window id, 0..NCORES*NW-1
    counts = np.bincount(gwin, minlength=NCORES * NW)
    nchw = int(np.ceil(counts.max() / 128))
    nchunk = NW * nchw
    slots_per_core = nchunk * 128

    order = np.argsort(gwin, kind="stable")
    starts = np.zeros(NCORES * NW, np.int64)
    starts[1:] = np.cumsum(counts)[:-1]

    s_src = src[order]
    s_dst = dst[order]
    s_attr = attr[order]
    s_gwin = gwin[order]
    j = np.arange(E) - starts[s_gwin]            # rank within window
    s_core = s_gwin // NW
    s_w = s_gwin % NW
    slot = s_w * (nchw * 128) + j                # padded slot within core

    pairidx = np.zeros((NCORES, slots_per_core), np.int16)
    dstlp = np.zeros((NCORES, 128, nchunk), np.float32)
    attr_a = np.zeros((NCORES, 128, nchunk), np.float32)
    pairidx[s_core, slot] = (s_src // 2).astype(np.int16)
    p = slot % 128
    c = slot // 128
    dstlp[s_core, p, c] = ((s_dst - s_core * NPC - s_w * WIN)
                           + 128 * (s_src & 1)).astype(np.float32)
    attr_a[s_core, p, c] = s_attr

    idx_packed = np.empty((NCORES, 128, slots_per_core // 16), np.int16)
    for k in range(NCORES):
        col = 0
        for (w0, w1) in _window_groups():
            nidx = (w1 - w0) * nchw * 128
            blk = _pack_idx_block(pairidx[k, w0 * nchw * 128:
                                          w0 * nchw * 128 + nidx])
            idx_packed[k, :, col:col + nidx // 16] = blk
            col += nidx // 16
    return nchw, pairidx, dstlp, attr_a, idx_packed


def kernel(x, edge_index, edge_attr, batch, W, b, gamma, beta):
    from concourse.bass_utils import run_bass_kernel_spmd

    x = np.asarray(x, dtype=np.float32)
    edge_index = np.asarray(edge_index)
    edge_attr = np.asarray(edge_attr, dtype=np.float32)
    W = np.asarray(W, dtype=np.float32)
    gamma = np.asarray(gamma, dtype=np.float32)
    beta = np.asarray(beta, dtype=np.float32)

    nchw, pairidx, dstlp, attr_a, idx_packed = _prep(x, edge_index, edge_attr)

    if nchw not in _build_cache:
        _build_cache[nchw] = _build(nchw)
    nc = _build_cache[nchw]

    table = np.ascontiguousarray(x.reshape(N // 2, 2 * D)).astype(_BF16)
    iota = np.tile(np.arange(2 * D, dtype=np.float32), (128, 1))
    wt = W.astype(_BF16)
    gamma_c = gamma.reshape(128, 1)
    beta_c = beta.reshape(128, 1)

    in_maps = []
    for k in range(NCORES):
        in_maps.append({
            "table": table,
            "idxs": np.ascontiguousarray(idx_packed[k]),
            "dstlp": np.ascontiguousarray(dstlp[k]),
            "attr": np.ascontiguousarray(attr_a[k]),
            "iota": iota,
            "wt": wt,
            "gamma": gamma_c,
            "beta": beta_c,
        })

    res = run_bass_kernel_spmd(nc, in_maps, list(range(NCORES)))
    out = np.empty((N, D), np.float32)
    for k in range(NCORES):
        out[k * NPC:(k + 1) * NPC, :] = res.results[k]["outT"].T
    return (out, edge_index, edge_attr, np.asarray(batch))


# revision 4
# speedup vs baseline: 1.5583x; 1.5583x over previous
"""Distributed Trainium2 (8 NeuronCores) kernel for a GCN layer:

    xw  = x @ W
    msg = edge_attr[:,None] * xw[src]
    agg = segment_sum(msg, dst, N) + b
    out = relu(gamma * (agg-mean)/sqrt(var+eps) + beta)   (BatchNorm, training stats)

Strategy (dst nodes sharded across 8 cores, x replicated to every core):
  - The W transform commutes with the weighted scatter-add: each core
    aggregates raw x features per dst node (aggx[n,:] = sum_e attr_e *
    x[src_e,:]) and applies W once per 128-node window (agg = aggx @ W).
  - Nodes are assigned to 8*63 windows by degree-balanced snake packing
    (host relabels nodes, un-permutes the output), so every window holds
    <=128 nodes and almost exactly E/504 edges -> minimal chunk padding.
  - Per 128-edge chunk the scatter-add is a matmul with a weighted one-hot
    A[e, col] = attr_e at col = pos(dst_e) + 128*(src_e & 1); A is built on
    the host and streamed from DRAM (the DMA port is separate from the
    engine ports, so this is free compared to building one-hots on the
    vector engine, which contends with the gather's descriptor generation
    for the shared GpSimd/DVE SBUF port).
  - The x[src] gather runs on the POOL dma_gather engine.  Rows are node
    PAIRS (256 feats, bf16): the gather index src//2 fits the int16 index
    format, and the even/odd selection is folded into A's column offset;
    the two pair halves feed two accumulating matmuls per chunk.
  - The post-aggregation bias b cancels inside BatchNorm and is dropped.
    BN statistics live with features on partitions: per-window sums/sumsq
    via scalar-engine accum_out, cross-core AllReduce of 1KB, then one
    fused scale+shift+ReLU activation per slab.  The output stays
    transposed [128 feat, nodes] on device; the host transposes and
    un-permutes.
"""

import sys

for _p in ("/opt/trn_rl_repo",):
    if _p not in sys.path:
        sys.path.insert(0, _p)

import numpy as np
import ml_dtypes

N = 64000
E = 640000
D = 128
NCORES = 8
WIN = 128                  # dst-window width (PSUM partitions)
NW = 63                    # windows per core
NBINS = NCORES * NW        # 504 node bins
GW = 4                     # windows per dma_gather group
BN_EPS = 1e-5

_BF16 = ml_dtypes.bfloat16
_build_cache = {}


def _window_groups():
    groups = []
    w0 = 0
    while w0 < NW:
        groups.append((w0, min(w0 + GW, NW)))
        w0 += GW
    return groups


def _pack_idx_block(idx_i16):
    """Wrap a gather's int16 index list: idx i -> partition i%16, col i//16,
    replicated across the 8 blocks of 16 partitions (Q7 core replicas)."""
    n = idx_i16.shape[0]
    block = idx_i16.reshape(n // 16, 16).T
    return np.tile(block, (8, 1))


def _build(nchw):
    import concourse.bacc as bacc
    import concourse.tile as tile
    from concourse import mybir

    nchunk = NW * nchw                 # chunks per core
    slots = nchunk * 128               # padded edge slots per core
    idxcols = slots // 16
    acols = nchw * 2 * D               # A-matrix columns per window

    nc = bacc.Bacc("TRN2", target_bir_lowering=False, debug=False,
                   enable_asserts=False, num_devices=NCORES,
                   num_swdge_queues=4)

    t_table = nc.dram_tensor("table", [N // 2, 2 * D], mybir.dt.bfloat16,
                             kind="ExternalInput")
    t_idxs = nc.dram_tensor("idxs", [128, idxcols], mybir.dt.int16,
                            kind="ExternalInput")
    t_amat = nc.dram_tensor("amat", [NW, 128, acols], mybir.dt.bfloat16,
                            kind="ExternalInput")
    t_w = nc.dram_tensor("wt", [D, D], mybir.dt.bfloat16, kind="ExternalInput")
    t_gamma = nc.dram_tensor("gamma", [128, 1], mybir.dt.float32,
                             kind="ExternalInput")
    t_beta = nc.dram_tensor("beta", [128, 1], mybir.dt.float32,
                            kind="ExternalInput")
    t_out = nc.dram_tensor("outT", [128, NW * WIN], mybir.dt.float32,
                           kind="ExternalOutput")
    t_ccin = nc.dram_tensor("cc_in", [128, 2], mybir.dt.float32)
    t_ccout = nc.dram_tensor("cc_out", [128, 2], mybir.dt.float32,
                             addr_space="Shared")

    groups = _window_groups()

    with tile.TileContext(nc) as tc:
        with (
            tc.tile_pool(name="const", bufs=1) as constp,
            tc.tile_pool(name="agg", bufs=1) as aggp,
            tc.tile_pool(name="xg", bufs=2) as xgp,
            tc.tile_pool(name="amat", bufs=3) as ap_,
            tc.tile_pool(name="axt", bufs=2) as axtp,
            tc.tile_pool(name="stat", bufs=1) as statp,
            tc.tile_pool(name="psA", bufs=2, space="PSUM") as psA,
            tc.tile_pool(name="psO", bufs=2, space="PSUM") as psO,
        ):
            w_t = constp.tile([D, D], mybir.dt.bfloat16)
            nc.sync.dma_start(out=w_t[:], in_=t_w.ap())
            gamma_t = constp.tile([128, 1], mybir.dt.float32)
            nc.sync.dma_start(out=gamma_t[:], in_=t_gamma.ap())
            beta_t = constp.tile([128, 1], mybir.dt.float32)
            nc.sync.dma_start(out=beta_t[:], in_=t_beta.ap())
            idx_t = constp.tile([128, idxcols], mybir.dt.int16)
            nc.sync.dma_start(out=idx_t[:], in_=t_idxs.ap())

            agg_sb = aggp.tile([128, NW * WIN], mybir.dt.float32)
            out_sb = aggp.tile([128, NW * WIN], mybir.dt.float32)
            sums_t = statp.tile([128, NW], mybir.dt.float32)
            sumsq_t = statp.tile([128, NW], mybir.dt.float32)
            sq_scr = statp.tile([128, WIN], mybir.dt.float32)
            zero1 = statp.tile([128, 1], mybir.dt.float32)
            nc.vector.memset(zero1[:], 0.0)

            for gi, (w0, w1) in enumerate(groups):
                gchunks = (w1 - w0) * nchw
                nidx = gchunks * 128
                xg = xgp.tile([128, GW * nchw, 2 * D], mybir.dt.bfloat16,
                              tag="xg")
                nc.gpsimd.dma_gather(
                    out_ap=xg[:, :gchunks, :],
                    in_ap=t_table.ap(),
                    idxs_ap=idx_t[:, w0 * nchw * 8:w1 * nchw * 8],
                    num_idxs=nidx,
                    num_idxs_reg=nidx,
                    elem_size=2 * D,
                    queue_num=gi % 4,
                    single_packet=False,
                )
                for w in range(w0, w1):
                    a_sb = ap_.tile([128, acols], mybir.dt.bfloat16,
                                    tag="amat")
                    nc.sync.dma_start(out=a_sb[:], in_=t_amat.ap()[w])
                    ps_aggx = psA.tile([128, WIN], mybir.dt.float32)
                    for c in range(nchw):
                        s = (w - w0) * nchw + c
                        nc.tensor.matmul(
                            out=ps_aggx[:],
                            lhsT=xg[:, s, 0:D],
                            rhs=a_sb[:, c * 2 * D:c * 2 * D + WIN],
                            start=(c == 0), stop=False,
                        )
                        nc.tensor.matmul(
                            out=ps_aggx[:],
                            lhsT=xg[:, s, D:2 * D],
                            rhs=a_sb[:, c * 2 * D + D:c * 2 * D + D + WIN],
                            start=False, stop=(c == nchw - 1),
                        )
                    aggx_bf = axtp.tile([128, WIN], mybir.dt.bfloat16,
                                        tag="axt")
                    nc.vector.tensor_copy(out=aggx_bf[:], in_=ps_aggx[:])
                    ps_out = psO.tile([128, WIN], mybir.dt.float32)
                    nc.tensor.matmul(out=ps_out[:], lhsT=w_t[:],
                                     rhs=aggx_bf[:], start=True, stop=True)
                    nc.scalar.activation(
                        out=agg_sb[:, w * WIN:(w + 1) * WIN],
                        in_=ps_out[:],
                        func=mybir.ActivationFunctionType.Copy,
                        accum_out=sums_t[:, w:w + 1],
                    )
                    nc.scalar.activation(
                        out=sq_scr[:],
                        in_=ps_out[:],
                        func=mybir.ActivationFunctionType.Square,
                        bias=zero1[:, 0:1],
                        accum_out=sumsq_t[:, w:w + 1],
                    )

            # ---- BN statistics: local reduce, cross-core all-reduce ----
            stat2 = statp.tile([128, 2], mybir.dt.float32)
            nc.vector.reduce_sum(stat2[:, 0:1], sums_t[:],
                                 axis=mybir.AxisListType.X)
            nc.vector.reduce_sum(stat2[:, 1:2], sumsq_t[:],
                                 axis=mybir.AxisListType.X)
            nc.sync.dma_start(out=t_ccin.ap(), in_=stat2[:])
            nc.gpsimd.collective_compute(
                "AllReduce", mybir.AluOpType.add,
                replica_groups=[list(range(NCORES))],
                ins=[t_ccin.ap().opt()],
                outs=[t_ccout.ap().opt()],
            )
            tot = statp.tile([128, 2], mybir.dt.float32)
            nc.sync.dma_start(out=tot[:], in_=t_ccout.ap())

            mean = statp.tile([128, 1], mybir.dt.float32)
            nc.vector.tensor_scalar_mul(mean[:], tot[:, 0:1], 1.0 / N)
            ex2 = statp.tile([128, 1], mybir.dt.float32)
            nc.vector.tensor_scalar_mul(ex2[:], tot[:, 1:2], 1.0 / N)
            msq = statp.tile([128, 1], mybir.dt.float32)
            nc.vector.tensor_tensor(out=msq[:], in0=mean[:], in1=mean[:],
                                    op=mybir.AluOpType.mult)
            var = statp.tile([128, 1], mybir.dt.float32)
            nc.vector.tensor_tensor(out=var[:], in0=ex2[:], in1=msq[:],
                                    op=mybir.AluOpType.subtract)
            var_eps = statp.tile([128, 1], mybir.dt.float32)
            nc.vector.tensor_scalar_add(var_eps[:], var[:], BN_EPS)
            sd = statp.tile([128, 1], mybir.dt.float32)
            nc.scalar.activation(out=sd[:], in_=var_eps[:],
                                 func=mybir.ActivationFunctionType.Sqrt,
                                 bias=zero1[:, 0:1])
            rinv = statp.tile([128, 1], mybir.dt.float32)
            nc.vector.reciprocal(rinv[:], sd[:])
            scale = statp.tile([128, 1], mybir.dt.float32)
            nc.vector.tensor_tensor(out=scale[:], in0=gamma_t[:], in1=rinv[:],
                                    op=mybir.AluOpType.mult)
            mscale = statp.tile([128, 1], mybir.dt.float32)
            nc.vector.tensor_tensor(out=mscale[:], in0=mean[:], in1=scale[:],
                                    op=mybir.AluOpType.mult)
            shift = statp.tile([128, 1], mybir.dt.float32)
            nc.vector.tensor_tensor(out=shift[:], in0=beta_t[:], in1=mscale[:],
                                    op=mybir.AluOpType.subtract)

            # ---- normalize + affine + relu, in slabs ----
            slab = 1008
            for s0 in range(0, NW * WIN, slab):
                s1 = min(s0 + slab, NW * WIN)
                nc.scalar.activation(
                    out=out_sb[:, s0:s1],
                    in_=agg_sb[:, s0:s1],
                    func=mybir.ActivationFunctionType.Relu,
                    scale=scale[:, 0:1],
                    bias=shift[:, 0:1],
                )
            nc.sync.dma_start(out=t_out.ap(), in_=out_sb[:])

    nc.compile()
    return nc


def _prep(x, edge_index, edge_attr):
    """Host-side binning/pack. Returns (nchw, idx_packed, amat, node_core,
    node_col) where node_core/node_col give each original node's position
    in its core's transposed output."""
    src = edge_index[0].astype(np.int64)
    dst = edge_index[1].astype(np.int64)
    attr = edge_attr.astype(np.float32)

    # degree-balanced snake packing of nodes into NBINS windows
    deg = np.bincount(dst, minlength=N)
    order = np.argsort(-deg, kind="stable")
    bin_of = np.empty(N, np.int64)
    for r in range((N + NBINS - 1) // NBINS):
        idx = order[r * NBINS:(r + 1) * NBINS]
        if r % 2 == 0:
            bin_of[idx] = np.arange(len(idx))
        else:
            bin_of[idx] = NBINS - 1 - np.arange(len(idx))
    binsum = np.bincount(bin_of, weights=deg.astype(np.float64),
                         minlength=NBINS).astype(np.int64)
    bincnt = np.bincount(bin_of, minlength=NBINS)
    assert bincnt.max() <= WIN
    nchw = max(int(np.ceil(binsum.max() / 128)), 1)

    # position of each node within its bin (ascending original id)
    nodesort = np.lexsort((np.arange(N), bin_of))
    pos_of = np.empty(N, np.int64)
    bstart = np.zeros(NBINS, np.int64)
    bstart[1:] = np.cumsum(bincnt)[:-1]
    pos_of[nodesort] = np.arange(N) - bstart[bin_of[nodesort]]
    node_core = bin_of // NW
    node_col = (bin_of % NW) * WIN + pos_of

    # edge -> (core, window, slot)
    e_bin = bin_of[dst]
    e_core = e_bin // NW
    e_w = e_bin % NW
    estart = np.zeros(NBINS, np.int64)
    estart[1:] = np.cumsum(binsum)[:-1]
    eorder = np.argsort(e_bin, kind="stable")
    j = np.empty(E, np.int64)
    j[eorder] = np.arange(E) - estart[e_bin[eorder]]   # rank within window

    nchunk = NW * nchw
    slots_per_core = nchunk * 128
    ec = j // 128                                      # chunk within window
    ep = j % 128                                       # partition

    pairidx = np.zeros((NCORES, slots_per_core), np.int16)
    pairidx[e_core, (e_w * nchw + ec) * 128 + ep] = (src // 2).astype(np.int16)

    amat = np.zeros((NCORES, NW, 128, nchw * 2 * D), _BF16)
    acol = ec * (2 * D) + pos_of[dst] + D * (src & 1)
    amat[e_core, e_w, ep, acol] = attr

    idx_packed = np.empty((NCORES, 128, slots_per_core // 16), np.int16)
    for k in range(NCORES):
        col = 0
        for (w0, w1) in _window_groups():
            nidx = (w1 - w0) * nchw * 128
            blk = _pack_idx_block(pairidx[k, w0 * nchw * 128:
                                          w0 * nchw * 128 + nidx])
            idx_packed[k, :, col:col + nidx // 16] = blk
            col += nidx // 16
    return nchw, idx_packed, amat, node_core, node_col


def _make_in_maps(x, W, gamma, beta, idx_packed, amat):
    table = np.ascontiguousarray(x.reshape(N // 2, 2 * D)).astype(_BF16)
    wt = W.astype(_BF16)
    gamma_c = np.ascontiguousarray(gamma.reshape(128, 1))
    beta_c = np.ascontiguousarray(beta.reshape(128, 1))
    in_maps = []
    for k in range(NCORES):
        in_maps.append({
            "table": table,
            "idxs": np.ascontiguousarray(idx_packed[k]),
            "amat": np.ascontiguousarray(amat[k]),
            "wt": wt,
            "gamma": gamma_c,
            "beta": beta_c,
        })
    return in_maps


def kernel(x, edge_index, edge_attr, batch, W, b, gamma, beta):
    from concourse.bass_utils import run_bass_kernel_spmd

    x = np.asarray(x, dtype=np.float32)
    edge_index = np.asarray(edge_index)
    edge_attr = np.asarray(edge_attr, dtype=np.float32)
    W = np.asarray(W, dtype=np.float32)
    gamma = np.asarray(gamma, dtype=np.float32)
    beta = np.asarray(beta, dtype=np.float32)

    nchw, idx_packed, amat, node_core, node_col = _prep(
        x, edge_index, edge_attr)

    if nchw not in _build_cache:
        _build_cache[nchw] = _build(nchw)
    nc = _build_cache[nchw]

    in_maps = _make_in_maps(x, W, gamma, beta, idx_packed, amat)
    res = run_bass_kernel_spmd(nc, in_maps, list(range(NCORES)))

    out = np.empty((N, D), np.float32)
    for k in range(NCORES):
        sel = node_core == k
        out[sel, :] = res.results[k]["outT"][:, node_col[sel]].T
    return (out, edge_index, edge_attr, np.asarray(batch))
